# revision 1
# baseline (speedup 1.0000x reference)
"""Single-head causal attention (B=4, T=2048, C=1024, H=128) on 8 trn2 cores.

Sharding: data-parallel over (batch, query-half). core c -> batch c//2,
query group c%2. Query rows are split causally-balanced: group 0 owns rows
[0,512)+[1536,2048), group 1 owns [512,1536). The host permutes x rows so
each core's own 1024 query rows come first; the key order is permuted the
same way, which makes the causal block structure identical on every core
(SPMD single NEFF). The only per-core difference is a 2-float bias that
zeroes key blocks that are fully masked for that core (applied inside exp).

Math (per core, permuted coords): qT/kT/vT = W.T @ xT via PE with xT built
by PE 128x128 transposes; scores^T[s,t] = kT_blk.T @ qT; E = exp(s/32 + bias)
(ACT, reads PSUM); diagonal 128x512 triangle masks multiplied in on GPSIMD;
out^T accumulated as v.T @ E^T and denom row as ones.T @ E^T on PE; denom
replicated across partitions with a K=1 outer-product matmul; normalize,
PE-transpose back to [t,H], DMA out. Matmuls run as float32r (~1e-4 rel).
"""

import sys

if "/opt/trn_rl_repo" not in sys.path:
    sys.path.insert(0, "/opt/trn_rl_repo")

import numpy as np

B, T, C, H = 4, 2048, 1024, 128
P = 128
TJ = 512                 # t-block (free dim) size
NK = C // P              # 8 contraction chunks
TOWN = 1024              # own query rows per core
NJ = TOWN // TJ          # 2 query blocks per core
NEG = -1e30
INV_SCALE = 1.0 / 32.0   # C ** -0.5

# key-block sets per query block j' (see module docstring):
#  j'=0: blocks 0-3 diagonal, 8-11 biased (bias col 0), 4-7 & 12-15 skipped
#  j'=1: 0-3 & 8-11 full, 4-7 diagonal, 12-15 biased (bias col 1)
SSET = {
    0: [0, 1, 2, 3, 8, 9, 10, 11],
    1: list(range(16)),
}
DIAG_BASE = {0: 0, 1: 4}          # diag blocks: [base, base+4); mask M[sb-base]
BIAS_GROUP = {0: {8: 0, 9: 0, 10: 0, 11: 0}, 1: {12: 1, 13: 1, 14: 1, 15: 1}}

_CACHE = {}


def _build_nc():
    import concourse.bacc as bacc
    import concourse.mybir as mybir
    import concourse.tile as tile
    from concourse.masks import make_identity

    f32 = mybir.dt.float32
    f32r = mybir.dt.float32r

    nc = bacc.Bacc("TRN2", target_bir_lowering=False, debug=False, num_devices=8)

    x = nc.dram_tensor("x", [T, C], f32, kind="ExternalInput").ap()
    wq = nc.dram_tensor("wq", [C, H], f32, kind="ExternalInput").ap()
    wk = nc.dram_tensor("wk", [C, H], f32, kind="ExternalInput").ap()
    wv = nc.dram_tensor("wv", [C, H], f32, kind="ExternalInput").ap()
    sbias = nc.dram_tensor("sbias", [P, 2], f32, kind="ExternalInput").ap()
    out = nc.dram_tensor("out", [TOWN, H], f32, kind="ExternalOutput").ap()

    Exp = mybir.ActivationFunctionType.Exp

    with tile.TileContext(nc) as tc:
        with (
            tc.tile_pool(name="singles", bufs=1) as singles,
            tc.tile_pool(name="xn", bufs=8) as xn_pool,
            tc.tile_pool(name="etile", bufs=3) as e_pool,
            tc.tile_pool(name="stage", bufs=2) as stage,
            tc.tile_pool(name="pp_s2", bufs=2, space="PSUM") as pp_s2,
            tc.tile_pool(name="pp_od", bufs=2, space="PSUM") as pp_od,
        ):
            # ---- startup: constants the transposes need, then weights ----
            ident = singles.tile([P, P], f32, tag="ident")
            make_identity(nc, ident)
            ones_f = singles.tile([P, 1], f32, tag="ones_f")
            nc.gpsimd.memset(ones_f, 1.0)
            ones_col = singles.tile([P, 1], f32r, tag="ones_col")
            nc.vector.tensor_copy(out=ones_col, in_=ones_f)
            ones_row = singles.tile([1, P], f32, tag="ones_row")
            nc.gpsimd.memset(ones_row, 1.0)
            warm = singles.tile([P, 1], f32, tag="warm")
            nc.scalar.activation(out=warm, in_=ones_f, func=Exp)
            sbias_sb = singles.tile([P, 2], f32, tag="sbias")
            nc.sync.dma_start(out=sbias_sb, in_=sbias)
            w_sb = {}
            for name, w in (("wq", wq), ("wk", wk), ("wv", wv)):
                tf = singles.tile([P, NK, H], f32, tag=f"{name}f",
                                  name=f"wf_{name}")
                nc.scalar.dma_start(out=tf,
                                    in_=w.rearrange("(k p) h -> p k h", p=P))
                t = singles.tile([P, NK, H], f32r, tag=name, name=f"w_{name}")
                nc.vector.tensor_copy(out=t, in_=tf)
                w_sb[name] = t

            # alternate PSUM->SBUF copies between DVE and ACT (setup phases
            # only; during attention ACT is reserved for exp)
            cp_state = [0]

            def copy_psum(dst, src):
                if cp_state[0] % 2 == 0:
                    nc.vector.tensor_copy(out=dst, in_=src)
                else:
                    nc.scalar.copy(out=dst, in_=src)
                cp_state[0] += 1

            xT = {}
            qT = {}
            kT = {}
            vN = {}

            def load_transpose_project(J):
                """DMA 4 row-blocks of x, transpose to xT[J], project q/k/v."""
                xts = []
                for di in range(4):
                    i = 4 * J + di
                    xt = xn_pool.tile([P, C], f32, tag="xn")
                    eng = nc.sync if (i % 2 == 0) else nc.scalar
                    eng.dma_start(out=xt, in_=x[P * i:P * (i + 1), :])
                    xts.append(xt)
                xT[J] = singles.tile([P, NK, TJ], f32r, tag=f"xT{J}",
                                     name=f"xT{J}")
                for kp in range(0, NK, 2):  # pairs of c-chunks per psum slot
                    ps = pp_s2.tile([P, 2, TJ], f32, tag="s2")
                    for g in range(2):
                        for di in range(4):
                            nc.tensor.transpose(
                                ps[:, g, P * di:P * (di + 1)],
                                xts[di][:, P * (kp + g):P * (kp + g + 1)],
                                ident,
                            )
                    copy_psum(xT[J][:, kp:kp + 2, :], ps)

                # projections: k and v packed into one psum slot; q (J<NJ) and
                # the v-transpose in another.
                ps_kv = pp_s2.tile([P, 2, TJ], f32, tag="s2")
                for k in range(NK):
                    st, sp = (k == 0), (k == NK - 1)
                    nc.tensor.matmul(ps_kv[:, 0, :], w_sb["wk"][:, k, :],
                                     xT[J][:, k, :], start=st, stop=sp)
                    nc.tensor.matmul(ps_kv[:, 1, :], w_sb["wv"][:, k, :],
                                     xT[J][:, k, :], start=st, stop=sp)
                kT[J] = singles.tile([P, TJ], f32r, tag=f"kT{J}", name=f"kT{J}")
                copy_psum(kT[J], ps_kv[:, 0, :])
                vT = stage.tile([P, TJ], f32, tag="vT")
                copy_psum(vT, ps_kv[:, 1, :])

                ps_qv = pp_s2.tile([P, 2, TJ], f32, tag="s2")
                if J < NJ:
                    for k in range(NK):
                        nc.tensor.matmul(ps_qv[:, 0, :], w_sb["wq"][:, k, :],
                                         xT[J][:, k, :],
                                         start=(k == 0), stop=(k == NK - 1))
                    qT[J] = singles.tile([P, TJ], f32r, tag=f"qT{J}",
                                         name=f"qT{J}")
                    copy_psum(qT[J], ps_qv[:, 0, :])
                for di in range(4):
                    nc.tensor.transpose(
                        ps_qv[:, 1, P * di:P * (di + 1)],
                        vT[:, P * di:P * (di + 1)],
                        ident,
                    )
                vN[J] = singles.tile([P, 4, H], f32r, tag=f"vN{J}",
                                     name=f"vN{J}")
                copy_psum(vN[J], ps_qv[:, 1, :].rearrange("p (d h) -> p d h",
                                                          d=4))

            # diagonal masks M[d][r, u] = 1 if u >= r + 128*d else 0
            masks = []

            def build_masks():
                for d in range(4):
                    mf = stage.tile([P, TJ], f32, tag="maskf")
                    nc.gpsimd.memset(mf, 1.0)
                    nc.gpsimd.affine_select(
                        out=mf, in_=mf,
                        compare_op=mybir.AluOpType.is_ge,
                        fill=0.0,
                        base=-P * d,
                        pattern=[[1, TJ]],
                        channel_multiplier=-1,
                    )
                    m = singles.tile([P, TJ], f32r, tag=f"mask{d}",
                                     name=f"mask{d}")
                    nc.vector.tensor_copy(out=m, in_=mf)
                    masks.append(m)

            oT = {}
            denom = singles.tile([1, TOWN], f32, tag="denom")

            def attention(j):
                sset = SSET[j]
                ps_od = pp_od.tile([P, 2, TJ], f32, tag="od")
                nmm = len(sset)
                db = DIAG_BASE[j]

                def emit_scores(pair):
                    ps2 = pp_s2.tile([P, 2, TJ], f32, tag="s2")
                    for ri, sb in enumerate(pair):
                        nc.tensor.matmul(
                            ps2[:, ri, :],
                            kT[sb // 4][:, P * (sb % 4):P * (sb % 4 + 1)],
                            qT[j],
                            start=True, stop=True,
                        )
                    bg = BIAS_GROUP[j].get(pair[0])
                    bias = sbias_sb[:, bg:bg + 1] if bg is not None else 0.0
                    e2 = e_pool.tile([P, 2, TJ], f32r, tag="e2")
                    nc.scalar.activation(
                        out=e2, in_=ps2, func=Exp, scale=INV_SCALE, bias=bias,
                    )
                    for ri, sb in enumerate(pair):
                        if db <= sb < db + 4:
                            nc.vector.tensor_mul(
                                out=e2[:, ri, :], in0=e2[:, ri, :],
                                in1=masks[sb - db],
                            )
                    return e2

                def emit_av(pair, e2, mm):
                    for ri, sb in enumerate(pair):
                        st, sp = (mm == 0), (mm == nmm - 1)
                        nc.tensor.matmul(ps_od[:, 0, :],
                                         vN[sb // 4][:, sb % 4, :],
                                         e2[:, ri, :], start=st, stop=sp)
                        nc.tensor.matmul(ps_od[0:1, 1, :], ones_col,
                                         e2[:, ri, :], start=st, stop=sp)
                        mm += 1
                    return mm

                pairs = [sset[pi:pi + 2] for pi in range(0, nmm, 2)]
                mm = 0
                prev = None
                for pair in pairs:
                    e2 = emit_scores(pair)
                    if prev is not None:
                        mm = emit_av(prev[0], prev[1], mm)
                    prev = (pair, e2)
                mm = emit_av(prev[0], prev[1], mm)
                oT[j] = stage.tile([P, TJ], f32, tag=f"oT{j}", name=f"oT{j}")
                nc.vector.tensor_copy(out=oT[j], in_=ps_od[:, 0, :])
                nc.vector.tensor_copy(out=denom[0:1, TJ * j:TJ * (j + 1)],
                                      in_=ps_od[0:1, 1, :])

            recip = singles.tile([1, TOWN], f32, tag="recip")

            def out_phase(j):
                rj = recip[0:1, TJ * j:TJ * (j + 1)]
                nc.vector.reciprocal(out=rj,
                                     in_=denom[0:1, TJ * j:TJ * (j + 1)])
                ps = pp_s2.tile([P, 2, TJ], f32, tag="s2")
                nc.tensor.matmul(ps[:, 0, :], ones_row, rj,
                                 start=True, stop=True)
                otn = stage.tile([P, TJ], f32, tag="otn")
                nc.vector.tensor_mul(out=otn, in0=oT[j], in1=ps[:, 0, :])
                for di in range(4):
                    nc.tensor.transpose(
                        ps[:, 1, P * di:P * (di + 1)],
                        otn[:, P * di:P * (di + 1)],
                        ident,
                    )
                ob = stage.tile([P, 4, H], f32, tag="ob")
                nc.vector.tensor_copy(
                    out=ob, in_=ps[:, 1, :].rearrange("p (d h) -> p d h", d=4))
                nc.sync.dma_start(
                    out=out[TJ * j:TJ * (j + 1), :].rearrange(
                        "(d p) h -> p d h", p=P),
                    in_=ob,
                )

            # ---- emission order: J=0,2 -> attention j'=0 ‖ J=1,3 -> j'=1 ----
            load_transpose_project(0)
            load_transpose_project(2)
            build_masks()
            attention(0)
            out_phase(0)
            load_transpose_project(1)
            load_transpose_project(3)
            attention(1)
            out_phase(1)

    nc.compile()
    return nc


def _get_nc():
    if "nc" not in _CACHE:
        _CACHE["nc"] = _build_nc()
    return _CACHE["nc"]


def kernel(x, Wq, Wk, Wv, mask=None):
    from concourse.bass_utils import run_bass_kernel_spmd

    nc = _get_nc()
    x = np.asarray(x, dtype=np.float32)
    Wq = np.ascontiguousarray(np.asarray(Wq, dtype=np.float32))
    Wk = np.ascontiguousarray(np.asarray(Wk, dtype=np.float32))
    Wv = np.ascontiguousarray(np.asarray(Wv, dtype=np.float32))

    in_maps = []
    for c in range(8):
        b, g = c // 2, c % 2
        xb = x[b]
        if g == 0:
            xp = np.concatenate([xb[0:512], xb[1536:2048], xb[512:1536]],
                                axis=0)
            sb = np.array([NEG, 0.0], dtype=np.float32)
        else:
            xp = np.concatenate([xb[512:1536], xb[0:512], xb[1536:2048]],
                                axis=0)
            sb = np.array([0.0, NEG], dtype=np.float32)
        in_maps.append({
            "x": np.ascontiguousarray(xp),
            "wq": Wq, "wk": Wk, "wv": Wv,
            "sbias": np.ascontiguousarray(np.broadcast_to(sb, (P, 2))),
        })

    res = run_bass_kernel_spmd(nc, in_maps, core_ids=list(range(8)))

    out = np.empty((B, T, H), dtype=np.float32)
    for c, rmap in enumerate(res.results):
        b, g = c // 2, c % 2
        o = rmap["out"]
        if g == 0:
            out[b, 0:512] = o[0:512]
            out[b, 1536:2048] = o[512:1024]
        else:
            out[b, 512:1536] = o
    return out



# revision 6
# speedup vs baseline: 1.0721x; 1.0721x over previous
"""Single-head causal attention (B=4, T=2048, C=1024, H=128) on 8 trn2 cores.

The wall clock is dominated by the axon tunnel (up ~48 ms fixed + 52 MB/s,
down ~82 ms fixed + 55 MB/s per pull), so the kernel minimizes transferred
bytes and host pulls:

- Each core c (batch b=c//2, half g=c%2) receives ONLY its own 1024 rows of
  x[b], int8-quantized with a per-token bf16 scale (8 MB total for x), plus
  a [50,1024] bf16 side tensor: a 1/8 row-shard of [Wq|Wk|Wv] (48 rows), the
  transposed dequant scales (row 48), and 32 mask thresholds (row 49).
- Weights are reconstructed on device with an all-8 AllGather; K^T and V for
  the full 2048-token batch are exchanged between the two cores of a batch
  with a pair AllGather (device-to-device, off the tunnel).
- The causal mask pattern differs per core (keys stay in natural order), so
  mask tiles are built on device from a static iota ramp compared against
  per-core thresholds: for query block j and key 128-block sb, valid iff
  t - s >= 128*sb - 1024*g - 512*j.
- Math: dequant x to bf16 (DVE, per-partition scale); qT/kT/vT = W.T @ xT
  (PE, bf16); scores^T = kT_blk.T @ qT; E = exp(s/32) (ACT, PSUM->bf16) *
  mask; out^T += v.T @ E^T and denom += 1.E^T on PE; normalize, PE-transpose
  back.
- The output is int8-quantized on device with a per-token bf16 scale (abs-max
  reduce on DVE; the f32->int8 convert rounds half-even), scales packed as 16
  bitcast rows after the 1024 token rows, then all-8 AllGathered so the jit
  returns a replicated [8320,128] int8 array: one ~1 MB host pull instead of
  8 slow per-shard pulls; the host dequantizes to fp32.

The run path is a module-cached jax.jit(shard_map(bass_exec)) mirroring
concourse.bass_utils.run_bass_kernel_spmd's axon redirect, kept cached so
warm calls skip re-tracing, with no donated zero output buffers (every
output element is written by the kernel). Host-side quant/pack is plain
numpy with preallocated buffers (the host has a single CPU).
"""

import sys

if "/opt/trn_rl_repo" not in sys.path:
    sys.path.insert(0, "/opt/trn_rl_repo")

import numpy as np

B, T, C, H = 4, 2048, 1024, 128
P = 128
TOWN = 1024              # own tokens per core
NK = C // P              # 8 contraction chunks
TJ = 512                 # query block size
NJ = TOWN // TJ          # 2 query blocks
NSB = T // P             # 16 key 128-blocks
WROWS = 3 * H * P // TOWN  # 48 packed weight-shard rows
METAR = WROWS + 2        # 50 meta rows: weights, scales, thresholds
INV_SCALE = 1.0 / 32.0   # C ** -0.5

PAIRS = [[0, 1], [2, 3], [4, 5], [6, 7]]
ALL8 = [list(range(8))]

_CACHE = {}


def _build_nc():
    import concourse.bacc as bacc
    import concourse.mybir as mybir
    import concourse.tile as tile
    from concourse.masks import make_identity

    f32 = mybir.dt.float32
    bf16 = mybir.dt.bfloat16
    i32 = mybir.dt.int32
    i8 = mybir.dt.int8
    Exp = mybir.ActivationFunctionType.Exp

    nc = bacc.Bacc("TRN2", target_bir_lowering=False, debug=False, num_devices=8)

    xq = nc.dram_tensor("xq", [TOWN, C], i8, kind="ExternalInput").ap()
    meta = nc.dram_tensor("meta", [METAR, TOWN], bf16, kind="ExternalInput").ap()
    # out: per core 1024 int8 token rows + 16 rows of bf16 scales (bitcast)
    out = nc.dram_tensor("out", [8 * (TOWN + 16), H], i8,
                         kind="ExternalOutput").ap()

    with tile.TileContext(nc) as tc:
        with (
            tc.tile_pool(name="singles", bufs=1) as singles,
            tc.tile_pool(name="xn", bufs=8) as xn_pool,
            tc.tile_pool(name="xb", bufs=8) as xb_pool,
            tc.tile_pool(name="etile", bufs=3) as e_pool,
            tc.tile_pool(name="stage", bufs=2) as stage,
            tc.tile_pool(name="pp_mm", bufs=2, space="PSUM") as pp_mm,
            tc.tile_pool(name="pp_od", bufs=1, space="PSUM") as pp_od,
            tc.tile_pool(name="pp_tr", bufs=2, space="PSUM") as pp_tr,
            tc.tile_pool(name="dram", bufs=1, space="DRAM") as dram,
        ):
            # ---- constants ----
            ident = singles.tile([P, P], bf16, tag="ident")
            make_identity(nc, ident)
            ones_bf = singles.tile([P, 1], bf16, tag="ones_bf")
            nc.gpsimd.memset(ones_bf, 1.0)
            ones_row = singles.tile([1, P], f32, tag="ones_row")
            nc.gpsimd.memset(ones_row, 1.0)
            ramp_i = stage.tile([P, TJ], i32, tag="ramp_i")
            nc.gpsimd.iota(ramp_i, pattern=[[1, TJ]], base=0,
                           channel_multiplier=-1)
            ramp = singles.tile([P, TJ], f32, tag="ramp")
            nc.vector.tensor_copy(out=ramp, in_=ramp_i)
            warm_in = singles.tile([P, 1], f32, tag="warm_in")
            nc.gpsimd.memset(warm_in, 1.0)
            warm = singles.tile([P, 1], f32, tag="warm")
            nc.scalar.activation(out=warm, in_=warm_in, func=Exp)

            # alternate PSUM->SBUF copies between DVE and ACT (setup only)
            cp_state = [0]

            def copy_psum(dst, src):
                if cp_state[0] % 2 == 0:
                    nc.vector.tensor_copy(out=dst, in_=src)
                else:
                    nc.scalar.copy(out=dst, in_=src)
                cp_state[0] += 1

            # ---- weight shard -> AllGather -> SBUF ----
            wsh = dram.tile([WROWS, TOWN], bf16)
            nc.scalar.dma_start(out=wsh, in_=meta[0:WROWS, :])
            wall = dram.tile([8, P, 3 * H], bf16)
            nc.gpsimd.collective_compute(
                "AllGather", mybir.AluOpType.bypass,
                replica_groups=ALL8, ins=[wsh.opt()], outs=[wall.opt()],
            )
            w_sb = singles.tile([P, NK, 3 * H], bf16, tag="w_sb")
            for k in range(NK):
                eng = nc.sync if (k % 2 == 0) else nc.scalar
                eng.dma_start(out=w_sb[:, k, :], in_=wall[k])

            # ---- dequant scales: meta row 48, transposed layout ----
            scl_bf = stage.tile([P, 8], bf16, tag="scl_bf")
            nc.sync.dma_start(
                out=scl_bf,
                in_=meta[WROWS:WROWS + 1, :].rearrange("r (p i) -> (r p) i", p=P),
            )
            scl = singles.tile([P, 8], f32, tag="scl")
            nc.vector.tensor_copy(out=scl, in_=scl_bf)

            # ---- thresholds -> [P, 32] f32 via broadcast matmul ----
            thr_bf = stage.tile([1, NJ * NSB], bf16, tag="thr_bf")
            nc.sync.dma_start(out=thr_bf,
                              in_=meta[WROWS + 1:METAR, 0:NJ * NSB])
            thr_row = stage.tile([1, NJ * NSB], f32, tag="thr_row")
            nc.vector.tensor_copy(out=thr_row, in_=thr_bf)
            ps_thr = pp_mm.tile([P, 2, TJ], f32, tag="mm")
            nc.tensor.matmul(ps_thr[:, 0, 0:NJ * NSB], ones_row, thr_row,
                             start=True, stop=True)
            thr = singles.tile([P, NJ * NSB], f32, tag="thr")
            copy_psum(thr, ps_thr[:, 0, 0:NJ * NSB])

            # ---- mask tiles: M[j*16+sb] = (t - s >= thr) ----
            maskt = singles.tile([P, NJ * NSB, TJ], bf16, tag="maskt")
            for m in range(NJ * NSB):
                nc.vector.tensor_scalar(
                    out=maskt[:, m, :], in0=ramp, scalar1=thr[:, m:m + 1],
                    scalar2=None, op0=mybir.AluOpType.is_ge,
                )

            # ---- load own x (int8), dequant to bf16, transpose to xT ----
            xbf = []
            for i in range(8):
                xi = xn_pool.tile([P, C], i8, tag="xn")
                eng = nc.sync if (i % 2 == 0) else nc.scalar
                eng.dma_start(out=xi, in_=xq[P * i:P * (i + 1), :])
                xb = xb_pool.tile([P, C], bf16, tag="xb")
                nc.vector.tensor_scalar(
                    out=xb, in0=xi, scalar1=scl[:, i:i + 1], scalar2=None,
                    op0=mybir.AluOpType.mult,
                )
                xbf.append(xb)
            xT = singles.tile([P, NK, TOWN], bf16, tag="xT")
            for half in range(2):
                for kp in range(0, NK, 2):
                    ps = pp_tr.tile([P, 2, TJ], bf16, tag="tr")
                    for g2 in range(2):
                        k = kp + g2
                        for di in range(4):
                            nc.tensor.transpose(
                                ps[:, g2, P * di:P * (di + 1)],
                                xbf[4 * half + di][:, P * k:P * (k + 1)],
                                ident,
                            )
                    copy_psum(xT[:, kp:kp + 2, TJ * half:TJ * (half + 1)], ps)

            # ---- projections q/k/v for own tokens ----
            qT = singles.tile([P, TOWN], bf16, tag="qT")
            kT_own = singles.tile([P, TOWN], bf16, tag="kT_own")
            vN_own = singles.tile([P, NK, H], bf16, tag="vN_own")
            for tb in range(2):
                ts = slice(TJ * tb, TJ * (tb + 1))
                ps_kv = pp_mm.tile([P, 2, TJ], f32, tag="mm")
                for k in range(NK):
                    st, sp = (k == 0), (k == NK - 1)
                    nc.tensor.matmul(ps_kv[:, 0, :], w_sb[:, k, H:2 * H],
                                     xT[:, k, ts], start=st, stop=sp)
                    nc.tensor.matmul(ps_kv[:, 1, :], w_sb[:, k, 2 * H:3 * H],
                                     xT[:, k, ts], start=st, stop=sp)
                copy_psum(kT_own[:, ts], ps_kv[:, 0, :])
                vT_half = stage.tile([P, TJ], bf16, tag="vT")
                copy_psum(vT_half, ps_kv[:, 1, :])

                ps_q = pp_mm.tile([P, 2, TJ], f32, tag="mm")
                for k in range(NK):
                    st, sp = (k == 0), (k == NK - 1)
                    nc.tensor.matmul(ps_q[:, 0, :], w_sb[:, k, 0:H],
                                     xT[:, k, ts], start=st, stop=sp)
                copy_psum(qT[:, ts], ps_q[:, 0, :])

                ps_v = pp_tr.tile([P, 2, TJ], bf16, tag="tr")
                for di in range(4):
                    nc.tensor.transpose(
                        ps_v[:, 0, P * di:P * (di + 1)],
                        vT_half[:, P * di:P * (di + 1)],
                        ident,
                    )
                copy_psum(
                    vN_own[:, 4 * tb:4 * (tb + 1), :],
                    ps_v[:, 0, :].rearrange("p (d h) -> p d h", d=4),
                )

            # ---- pair AllGather of (kT, vN) ----
            kv_in = dram.tile([P, 2 * TOWN], bf16)
            nc.sync.dma_start(out=kv_in[:, 0:TOWN], in_=kT_own)
            nc.scalar.dma_start(
                out=kv_in[:, TOWN:2 * TOWN],
                in_=vN_own.rearrange("p d h -> p (d h)"),
            )
            kv_out = dram.tile([2, P, 2 * TOWN], bf16)
            nc.gpsimd.collective_compute(
                "AllGather", mybir.AluOpType.bypass,
                replica_groups=PAIRS, ins=[kv_in.opt()], outs=[kv_out.opt()],
            )
            kT = singles.tile([P, 2, TOWN], bf16, tag="kT")
            vN = singles.tile([P, 2, NK, H], bf16, tag="vN")
            for r in range(2):
                nc.sync.dma_start(out=kT[:, r, :], in_=kv_out[r, :, 0:TOWN])
                nc.scalar.dma_start(
                    out=vN[:, r, :, :].rearrange("p d h -> p (d h)"),
                    in_=kv_out[r, :, TOWN:2 * TOWN],
                )

            # ---- attention per query block ----
            oT = {}
            denom = singles.tile([1, TOWN], f32, tag="denom")

            def attention(j):
                ps_od = pp_od.tile([P, 2, TJ], f32, tag="od")
                nmm = NSB

                def emit_scores(pair):
                    ps2 = pp_mm.tile([P, 2, TJ], f32, tag="mm")
                    for ri, sb in enumerate(pair):
                        r, i = sb // NK, sb % NK
                        nc.tensor.matmul(
                            ps2[:, ri, :],
                            kT[:, r, P * i:P * (i + 1)],
                            qT[:, TJ * j:TJ * (j + 1)],
                            start=True, stop=True,
                        )
                    e2 = e_pool.tile([P, 2, TJ], bf16, tag="e2")
                    nc.scalar.activation(out=e2, in_=ps2, func=Exp,
                                         scale=INV_SCALE)
                    for ri, sb in enumerate(pair):
                        nc.vector.tensor_mul(
                            out=e2[:, ri, :], in0=e2[:, ri, :],
                            in1=maskt[:, NSB * j + sb, :],
                        )
                    return e2

                def emit_av(pair, e2, mm):
                    for ri, sb in enumerate(pair):
                        r, i = sb // NK, sb % NK
                        st, sp = (mm == 0), (mm == nmm - 1)
                        nc.tensor.matmul(ps_od[:, 0, :], vN[:, r, i, :],
                                         e2[:, ri, :], start=st, stop=sp)
                        nc.tensor.matmul(ps_od[0:1, 1, :], ones_bf,
                                         e2[:, ri, :], start=st, stop=sp)
                        mm += 1
                    return mm

                pairs = [(pi, pi + 1) for pi in range(0, NSB, 2)]
                mm = 0
                prev = None
                for pair in pairs:
                    e2 = emit_scores(pair)
                    if prev is not None:
                        mm = emit_av(prev[0], prev[1], mm)
                    prev = (pair, e2)
                mm = emit_av(prev[0], prev[1], mm)
                oT[j] = stage.tile([P, TJ], f32, tag=f"oT{j}", name=f"oT{j}")
                nc.vector.tensor_copy(out=oT[j], in_=ps_od[:, 0, :])
                nc.vector.tensor_copy(out=denom[0:1, TJ * j:TJ * (j + 1)],
                                      in_=ps_od[0:1, 1, :])

            recip = singles.tile([1, TOWN], f32, tag="recip")
            obounce = dram.tile([TOWN + 16, H], i8)
            sout = singles.tile([P, 8], bf16, tag="sout")

            def out_phase(j):
                rj = recip[0:1, TJ * j:TJ * (j + 1)]
                nc.vector.reciprocal(out=rj,
                                     in_=denom[0:1, TJ * j:TJ * (j + 1)])
                ps = pp_mm.tile([P, 2, TJ], f32, tag="mm")
                nc.tensor.matmul(ps[:, 0, :], ones_row, rj,
                                 start=True, stop=True)
                otn = stage.tile([P, TJ], bf16, tag="otn")
                nc.vector.tensor_mul(out=otn, in0=oT[j], in1=ps[:, 0, :])
                ps_t = pp_tr.tile([P, 2, TJ], bf16, tag="tr")
                for di in range(4):
                    nc.tensor.transpose(
                        ps_t[:, 0, P * di:P * (di + 1)],
                        otn[:, P * di:P * (di + 1)],
                        ident,
                    )
                ob = stage.tile([P, 4, H], bf16, tag="ob")
                nc.vector.tensor_copy(
                    out=ob,
                    in_=ps_t[:, 0, :].rearrange("p (d h) -> p d h", d=4))
                # int8-quantize per token (partition = token): scale=absmax/127
                am = stage.tile([P, 4], f32, tag="am")
                for di in range(4):
                    nc.vector.tensor_reduce(
                        out=am[:, di:di + 1], in_=ob[:, di, :],
                        axis=mybir.AxisListType.X, op=mybir.AluOpType.max,
                        apply_absolute_value=True)
                nc.vector.tensor_scalar(
                    out=am, in0=am, scalar1=1.0 / 127.0, scalar2=1e-30,
                    op0=mybir.AluOpType.mult, op1=mybir.AluOpType.max)
                sc_j = sout[:, 4 * j:4 * (j + 1)]
                nc.vector.tensor_copy(out=sc_j, in_=am)
                sc_f = stage.tile([P, 4], f32, tag="sc_f")
                nc.vector.tensor_copy(out=sc_f, in_=sc_j)
                inv = stage.tile([P, 4], f32, tag="inv")
                nc.vector.reciprocal(out=inv, in_=sc_f)
                qo = stage.tile([P, 4, H], i8, tag="qo")
                for di in range(4):
                    nc.vector.tensor_scalar(
                        out=qo[:, di, :], in0=ob[:, di, :],
                        scalar1=inv[:, di:di + 1], scalar2=None,
                        op0=mybir.AluOpType.mult)
                nc.sync.dma_start(
                    out=obounce[TJ * j:TJ * (j + 1), :].rearrange(
                        "(d p) h -> p d h", p=P),
                    in_=qo,
                )

            attention(0)
            out_phase(0)
            attention(1)
            out_phase(1)
            nc.scalar.dma_start(out=obounce[TOWN:TOWN + 16, :],
                                in_=sout.bitcast(i8))

            # ---- replicate outputs: all-8 AllGather -> out ----
            gout = dram.tile([8, TOWN + 16, H], i8)
            nc.gpsimd.collective_compute(
                "AllGather", mybir.AluOpType.bypass,
                replica_groups=ALL8, ins=[obounce.opt()], outs=[gout.opt()],
            )
            nc.sync.dma_start(
                out=out,
                in_=gout.rearrange("c t h -> (c t) h"),
            )

    nc.compile()
    return nc


def _get_nc():
    if "nc" not in _CACHE:
        _CACHE["nc"] = _build_nc()
    return _CACHE["nc"]


def _thresholds():
    """negc[c, m]: mask threshold per core c, combo m = 16*j + sb."""
    negc = np.zeros((8, NJ * NSB), dtype=np.float32)
    for c in range(8):
        g = c % 2
        for j in range(NJ):
            for sb in range(NSB):
                negc[c, NSB * j + sb] = 128 * sb - 1024 * g - 512 * j
    return negc


def _f32_to_bf16_u16(a):
    """Round-half-up fp32 -> bf16, returned as uint16 payload."""
    u = np.ascontiguousarray(a, dtype=np.float32).view(np.uint32)
    return ((u + 0x8000) >> 16).astype(np.uint16)


def _bf16_u16_to_f32(u):
    return (u.astype(np.uint32) << 16).view(np.float32)


def _get_packer():
    """Cached numpy int8 quantizer: per-token bf16 scales, preallocated
    buffers (host has a single CPU, so minimize passes/allocations)."""
    if "packer" in _CACHE:
        return _CACHE["packer"]

    tmp = np.empty((8, TOWN, C), np.float32)
    q = np.empty((8, TOWN, C), np.int8)
    s = np.empty((8, TOWN, 1), np.float32)
    lo = np.empty((8, TOWN, 1), np.float32)
    inv = np.empty((8, TOWN, 1), np.float32)

    def packer(x8):
        # per-row absmax as max(max(x), -min(x)): two reads, no 32MB write
        np.max(x8, axis=-1, keepdims=True, out=s)
        np.min(x8, axis=-1, keepdims=True, out=lo)
        np.negative(lo, out=lo)
        np.maximum(s, lo, out=s)
        np.divide(s, 127.0, out=s)
        np.maximum(s, 1e-30, out=s)
        u = s.view(np.uint32)
        np.bitwise_and(u + 0x8000, 0xFFFF0000, out=u)  # round scale to bf16
        np.divide(1.0, s, out=inv)
        np.multiply(x8, inv, out=tmp)
        np.rint(tmp, out=tmp)
        np.copyto(q, tmp, casting="unsafe")
        # transposed scale layout: scl_t[c, p*8 + i] = scale[c, 128*i + p]
        st = np.ascontiguousarray(
            (u >> 16).astype(np.uint16).reshape(8, 8, P).transpose(0, 2, 1)
        ).reshape(8, TOWN)
        return q, st

    _CACHE["packer"] = packer
    return packer


def _get_runner():
    """Cached jit(shard_map(bass_exec)) mirroring run_bass_kernel_spmd's
    axon redirect, without per-call re-tracing or donated zero outputs."""
    if "runner" in _CACHE:
        return _CACHE["runner"]

    import jax
    import concourse.mybir as mybir
    from concourse.bass2jax import (
        _bass_exec_p, install_neuronx_cc_hook, partition_id_tensor,
    )
    from jax.sharding import Mesh, PartitionSpec
    from jax.experimental.shard_map import shard_map

    nc = _get_nc()
    install_neuronx_cc_hook()

    partition_name = (nc.partition_id_tensor.name
                      if nc.partition_id_tensor else None)
    in_names, out_names, out_avals = [], [], []
    for alloc in nc.m.functions[0].allocations:
        if not isinstance(alloc, mybir.MemoryLocationSet):
            continue
        name = alloc.memorylocations[0].name
        if alloc.kind == "ExternalInput":
            if name != partition_name:
                in_names.append(name)
        elif alloc.kind == "ExternalOutput":
            out_names.append(name)
            out_avals.append(jax.core.ShapedArray(
                tuple(alloc.tensor_shape), mybir.dt.np(alloc.dtype)))
    assert sorted(in_names) == ["meta", "xq"] and out_names == ["out"], (
        in_names, out_names)
    n_params = len(in_names)
    in_names_all = list(in_names)
    if partition_name is not None:
        in_names_all.append(partition_name)

    def _body(*args):
        operands = list(args)
        if partition_name is not None:
            operands.append(partition_id_tensor())
        return tuple(_bass_exec_p.bind(
            *operands,
            out_avals=tuple(out_avals),
            in_names=tuple(in_names_all),
            out_names=tuple(out_names),
            lowering_input_output_aliases=(),
            sim_require_finite=True,
            sim_require_nnan=True,
            nc=nc,
        ))

    devices = jax.devices()[:8]
    assert len(devices) == 8, f"need 8 devices, have {len(jax.devices())}"
    mesh = Mesh(np.asarray(devices), ("core",))
    sharded = jax.jit(shard_map(
        _body, mesh=mesh,
        in_specs=(PartitionSpec("core"),) * n_params,
        out_specs=(PartitionSpec(),) * len(out_names),
        check_rep=False,
    ))
    _CACHE["runner"] = sharded
    _CACHE["runner_in_names"] = in_names
    return sharded


def kernel(x, Wq, Wk, Wv, mask=None):
    import ml_dtypes

    runner = _get_runner()
    packer = _get_packer()

    if "meta" not in _CACHE:
        meta = np.zeros((8, METAR, TOWN), dtype=np.uint16)
        meta[:, WROWS + 1, 0:NJ * NSB] = _f32_to_bf16_u16(_thresholds())
        _CACHE["meta"] = meta
    meta = _CACHE["meta"]

    x = np.ascontiguousarray(np.asarray(x, dtype=np.float32))
    q, st = packer(x.reshape(8, TOWN, TOWN))
    wall = np.concatenate(
        [np.asarray(Wq, np.float32), np.asarray(Wk, np.float32),
         np.asarray(Wv, np.float32)], axis=1)
    meta[:, 0:WROWS, :] = _f32_to_bf16_u16(wall).reshape(8, WROWS, TOWN)
    meta[:, WROWS, :] = st.view(np.uint16)

    args = {
        "xq": q.reshape(8 * TOWN, C),
        "meta": meta.reshape(8 * METAR, TOWN).view(ml_dtypes.bfloat16),
    }
    in_names = _CACHE["runner_in_names"]
    (out_arr,) = runner(*[args[n] for n in in_names])
    ob = np.asarray(out_arr).reshape(8, TOWN + 16, H)
    qo = ob[:, 0:TOWN, :].astype(np.float32)
    # scales: [8, 16, 128] int8 -> uint16 payload [8, 128, 8] -> per-token
    sc_u = np.ascontiguousarray(ob[:, TOWN:TOWN + 16, :]).reshape(
        8, 2048).view(np.uint16).reshape(8, P, 8)
    sc = _bf16_u16_to_f32(np.ascontiguousarray(
        sc_u.reshape(8, P, 2, 4).transpose(0, 2, 3, 1)).reshape(8, TOWN))
    return (qo * sc[:, :, None]).reshape(B, T, H)


# revision 7
# speedup vs baseline: 1.4241x; 1.3283x over previous
"""Single-head causal attention (B=4, T=2048, C=1024, H=128) on 8 trn2 cores.

The wall clock is dominated by the axon tunnel (up ~48 ms fixed + 52 MB/s,
down ~82 ms fixed + 55 MB/s per pull), so the kernel minimizes transferred
bytes and host pulls:

- Each core c (batch b=c//2, half g=c%2) receives ONLY its own 1024 rows of
  x[b], int8-quantized with a per-token bf16 scale (8 MB total for x), plus
  a [52,1024] int8 side tensor: a 1/8 row-shard of [Wq|Wk|Wv] int8-quantized
  with per-column bf16 scales (rows 0:48 + row 48), the transposed x dequant
  scales (rows 49:51), and 32 mask thresholds (row 51) — bf16 values bitcast
  into int8 rows.
- Weights are reconstructed on device with an all-8 AllGather and dequantized
  to bf16 (scale row broadcast across partitions via a 1-row PE matmul); K^T
  and V for the full 2048-token batch are exchanged between the two cores of
  a batch with a pair AllGather (device-to-device, off the tunnel).
- The causal mask pattern differs per core (keys stay in natural order), so
  mask tiles are built on device from a static iota ramp compared against
  per-core thresholds: for query block j and key 128-block sb, valid iff
  t - s >= 128*sb - 1024*g - 512*j.
- Math: dequant x to bf16 (DVE, per-partition scale); qT/kT/vT = W.T @ xT
  (PE, bf16); scores^T = kT_blk.T @ qT; E = exp(s/32) (ACT, PSUM->bf16) *
  mask; out^T += v.T @ E^T and denom += 1.E^T on PE; normalize, PE-transpose
  back.
- The output is int8-quantized on device with a per-token bf16 scale (abs-max
  reduce on DVE; the f32->int8 convert rounds half-even), scales packed as 16
  bitcast rows after the 1024 token rows, then all-8 AllGathered so the jit
  returns a replicated [8320,128] int8 array: one ~1 MB host pull instead of
  8 slow per-shard pulls; the host dequantizes to fp32.

The run path is a module-cached jax.jit(shard_map(bass_exec)) mirroring
concourse.bass_utils.run_bass_kernel_spmd's axon redirect, kept cached so
warm calls skip re-tracing, with no donated zero output buffers (every
output element is written by the kernel). Host-side quant/pack is plain
numpy with preallocated buffers (the host has a single CPU).
"""

import sys

if "/opt/trn_rl_repo" not in sys.path:
    sys.path.insert(0, "/opt/trn_rl_repo")

import numpy as np

B, T, C, H = 4, 2048, 1024, 128
P = 128
TOWN = 1024              # own tokens per core
NK = C // P              # 8 contraction chunks
TJ = 512                 # query block size
NJ = TOWN // TJ          # 2 query blocks
NSB = T // P             # 16 key 128-blocks
WROWS = 3 * H * P // TOWN  # 48 packed weight-shard rows (int8)
METAR = WROWS + 4        # 52 int8 meta rows: weights, wscale, xscales, thr
INV_SCALE = 1.0 / 32.0   # C ** -0.5

PAIRS = [[0, 1], [2, 3], [4, 5], [6, 7]]
ALL8 = [list(range(8))]

_CACHE = {}


def _build_nc():
    import concourse.bacc as bacc
    import concourse.mybir as mybir
    import concourse.tile as tile
    from concourse.masks import make_identity

    f32 = mybir.dt.float32
    bf16 = mybir.dt.bfloat16
    i32 = mybir.dt.int32
    i8 = mybir.dt.int8
    Exp = mybir.ActivationFunctionType.Exp

    nc = bacc.Bacc("TRN2", target_bir_lowering=False, debug=False, num_devices=8)

    xq = nc.dram_tensor("xq", [TOWN, C], i8, kind="ExternalInput").ap()
    # meta (int8): rows 0:48 weight shard int8; row 48 col scales (384 bf16
    # bitcast); rows 49:51 x dequant scales (1024 bf16); row 51 thresholds
    meta = nc.dram_tensor("meta", [METAR, TOWN], i8, kind="ExternalInput").ap()
    # out: per core 1024 int8 token rows + 16 rows of bf16 scales (bitcast)
    out = nc.dram_tensor("out", [8 * (TOWN + 16), H], i8,
                         kind="ExternalOutput").ap()

    with tile.TileContext(nc) as tc:
        with (
            tc.tile_pool(name="singles", bufs=1) as singles,
            tc.tile_pool(name="xn", bufs=8) as xn_pool,
            tc.tile_pool(name="xb", bufs=8) as xb_pool,
            tc.tile_pool(name="etile", bufs=3) as e_pool,
            tc.tile_pool(name="stage", bufs=2) as stage,
            tc.tile_pool(name="pp_mm", bufs=2, space="PSUM") as pp_mm,
            tc.tile_pool(name="pp_od", bufs=1, space="PSUM") as pp_od,
            tc.tile_pool(name="pp_tr", bufs=2, space="PSUM") as pp_tr,
            tc.tile_pool(name="dram", bufs=1, space="DRAM") as dram,
        ):
            # ---- constants ----
            ident = singles.tile([P, P], bf16, tag="ident")
            make_identity(nc, ident)
            ones_bf = singles.tile([P, 1], bf16, tag="ones_bf")
            nc.gpsimd.memset(ones_bf, 1.0)
            ones_row = singles.tile([1, P], f32, tag="ones_row")
            nc.gpsimd.memset(ones_row, 1.0)
            ramp_i = stage.tile([P, TJ], i32, tag="ramp_i")
            nc.gpsimd.iota(ramp_i, pattern=[[1, TJ]], base=0,
                           channel_multiplier=-1)
            ramp = singles.tile([P, TJ], f32, tag="ramp")
            nc.vector.tensor_copy(out=ramp, in_=ramp_i)
            warm_in = singles.tile([P, 1], f32, tag="warm_in")
            nc.gpsimd.memset(warm_in, 1.0)
            warm = singles.tile([P, 1], f32, tag="warm")
            nc.scalar.activation(out=warm, in_=warm_in, func=Exp)

            # alternate PSUM->SBUF copies between DVE and ACT (setup only)
            cp_state = [0]

            def copy_psum(dst, src):
                if cp_state[0] % 2 == 0:
                    nc.vector.tensor_copy(out=dst, in_=src)
                else:
                    nc.scalar.copy(out=dst, in_=src)
                cp_state[0] += 1

            # ---- int8 weight shard -> AllGather -> dequant to bf16 SBUF ----
            wsh = dram.tile([WROWS, TOWN], i8)
            nc.scalar.dma_start(out=wsh, in_=meta[0:WROWS, :])
            wall = dram.tile([8, P, 3 * H], i8)
            nc.gpsimd.collective_compute(
                "AllGather", mybir.AluOpType.bypass,
                replica_groups=ALL8, ins=[wsh.opt()], outs=[wall.opt()],
            )
            w_i8 = singles.tile([P, NK, 3 * H], i8, tag="w_i8")
            for k in range(NK):
                eng = nc.sync if (k % 2 == 0) else nc.scalar
                eng.dma_start(out=w_i8[:, k, :], in_=wall[k])
            # per-column scales: broadcast [1,384] across partitions via PE
            wsc_bf = stage.tile([1, 3 * H], bf16, tag="wsc_bf")
            nc.sync.dma_start(out=wsc_bf,
                              in_=meta[WROWS:WROWS + 1, 0:6 * H].bitcast(bf16))
            wsc_f = stage.tile([1, 3 * H], f32, tag="wsc_f")
            nc.vector.tensor_copy(out=wsc_f, in_=wsc_bf)
            ps_w = pp_mm.tile([P, 2, TJ], f32, tag="mm")
            nc.tensor.matmul(ps_w[:, 0, 0:3 * H], ones_row, wsc_f,
                             start=True, stop=True)
            wbc = singles.tile([P, 3 * H], bf16, tag="wbc")
            copy_psum(wbc, ps_w[:, 0, 0:3 * H])
            w_sb = singles.tile([P, NK, 3 * H], bf16, tag="w_sb")
            for k in range(NK):
                nc.vector.tensor_copy(out=w_sb[:, k, :], in_=w_i8[:, k, :])
                nc.vector.tensor_mul(out=w_sb[:, k, :], in0=w_sb[:, k, :],
                                     in1=wbc)

            # ---- x dequant scales: meta rows 49:51, transposed layout ----
            scl_bf = stage.tile([P, 8], bf16, tag="scl_bf")
            for r in range(2):
                nc.sync.dma_start(
                    out=scl_bf[64 * r:64 * (r + 1), :],
                    in_=meta[WROWS + 1 + r:WROWS + 2 + r, :].bitcast(
                        bf16).rearrange("r (p i) -> (r p) i", p=64),
                )
            scl = singles.tile([P, 8], f32, tag="scl")
            nc.vector.tensor_copy(out=scl, in_=scl_bf)

            # ---- thresholds -> [P, 32] f32 via broadcast matmul ----
            thr_bf = stage.tile([1, NJ * NSB], bf16, tag="thr_bf")
            nc.sync.dma_start(
                out=thr_bf,
                in_=meta[WROWS + 3:METAR, 0:2 * NJ * NSB].bitcast(bf16))
            thr_row = stage.tile([1, NJ * NSB], f32, tag="thr_row")
            nc.vector.tensor_copy(out=thr_row, in_=thr_bf)
            ps_thr = pp_mm.tile([P, 2, TJ], f32, tag="mm")
            nc.tensor.matmul(ps_thr[:, 0, 0:NJ * NSB], ones_row, thr_row,
                             start=True, stop=True)
            thr = singles.tile([P, NJ * NSB], f32, tag="thr")
            copy_psum(thr, ps_thr[:, 0, 0:NJ * NSB])

            # ---- mask tiles: M[j*16+sb] = (t - s >= thr) ----
            maskt = singles.tile([P, NJ * NSB, TJ], bf16, tag="maskt")
            for m in range(NJ * NSB):
                nc.vector.tensor_scalar(
                    out=maskt[:, m, :], in0=ramp, scalar1=thr[:, m:m + 1],
                    scalar2=None, op0=mybir.AluOpType.is_ge,
                )

            # ---- load own x (int8), dequant to bf16, transpose to xT ----
            xbf = []
            for i in range(8):
                xi = xn_pool.tile([P, C], i8, tag="xn")
                eng = nc.sync if (i % 2 == 0) else nc.scalar
                eng.dma_start(out=xi, in_=xq[P * i:P * (i + 1), :])
                xb = xb_pool.tile([P, C], bf16, tag="xb")
                nc.vector.tensor_scalar(
                    out=xb, in0=xi, scalar1=scl[:, i:i + 1], scalar2=None,
                    op0=mybir.AluOpType.mult,
                )
                xbf.append(xb)
            xT = singles.tile([P, NK, TOWN], bf16, tag="xT")
            for half in range(2):
                for kp in range(0, NK, 2):
                    ps = pp_tr.tile([P, 2, TJ], bf16, tag="tr")
                    for g2 in range(2):
                        k = kp + g2
                        for di in range(4):
                            nc.tensor.transpose(
                                ps[:, g2, P * di:P * (di + 1)],
                                xbf[4 * half + di][:, P * k:P * (k + 1)],
                                ident,
                            )
                    copy_psum(xT[:, kp:kp + 2, TJ * half:TJ * (half + 1)], ps)

            # ---- projections q/k/v for own tokens ----
            qT = singles.tile([P, TOWN], bf16, tag="qT")
            kT_own = singles.tile([P, TOWN], bf16, tag="kT_own")
            vN_own = singles.tile([P, NK, H], bf16, tag="vN_own")
            for tb in range(2):
                ts = slice(TJ * tb, TJ * (tb + 1))
                ps_kv = pp_mm.tile([P, 2, TJ], f32, tag="mm")
                for k in range(NK):
                    st, sp = (k == 0), (k == NK - 1)
                    nc.tensor.matmul(ps_kv[:, 0, :], w_sb[:, k, H:2 * H],
                                     xT[:, k, ts], start=st, stop=sp)
                    nc.tensor.matmul(ps_kv[:, 1, :], w_sb[:, k, 2 * H:3 * H],
                                     xT[:, k, ts], start=st, stop=sp)
                copy_psum(kT_own[:, ts], ps_kv[:, 0, :])
                vT_half = stage.tile([P, TJ], bf16, tag="vT")
                copy_psum(vT_half, ps_kv[:, 1, :])

                ps_q = pp_mm.tile([P, 2, TJ], f32, tag="mm")
                for k in range(NK):
                    st, sp = (k == 0), (k == NK - 1)
                    nc.tensor.matmul(ps_q[:, 0, :], w_sb[:, k, 0:H],
                                     xT[:, k, ts], start=st, stop=sp)
                copy_psum(qT[:, ts], ps_q[:, 0, :])

                ps_v = pp_tr.tile([P, 2, TJ], bf16, tag="tr")
                for di in range(4):
                    nc.tensor.transpose(
                        ps_v[:, 0, P * di:P * (di + 1)],
                        vT_half[:, P * di:P * (di + 1)],
                        ident,
                    )
                copy_psum(
                    vN_own[:, 4 * tb:4 * (tb + 1), :],
                    ps_v[:, 0, :].rearrange("p (d h) -> p d h", d=4),
                )

            # ---- pair AllGather of (kT, vN) ----
            kv_in = dram.tile([P, 2 * TOWN], bf16)
            nc.sync.dma_start(out=kv_in[:, 0:TOWN], in_=kT_own)
            nc.scalar.dma_start(
                out=kv_in[:, TOWN:2 * TOWN],
                in_=vN_own.rearrange("p d h -> p (d h)"),
            )
            kv_out = dram.tile([2, P, 2 * TOWN], bf16)
            nc.gpsimd.collective_compute(
                "AllGather", mybir.AluOpType.bypass,
                replica_groups=PAIRS, ins=[kv_in.opt()], outs=[kv_out.opt()],
            )
            kT = singles.tile([P, 2, TOWN], bf16, tag="kT")
            vN = singles.tile([P, 2, NK, H], bf16, tag="vN")
            for r in range(2):
                nc.sync.dma_start(out=kT[:, r, :], in_=kv_out[r, :, 0:TOWN])
                nc.scalar.dma_start(
                    out=vN[:, r, :, :].rearrange("p d h -> p (d h)"),
                    in_=kv_out[r, :, TOWN:2 * TOWN],
                )

            # ---- attention per query block ----
            oT = {}
            denom = singles.tile([1, TOWN], f32, tag="denom")

            def attention(j):
                ps_od = pp_od.tile([P, 2, TJ], f32, tag="od")
                nmm = NSB

                def emit_scores(pair):
                    ps2 = pp_mm.tile([P, 2, TJ], f32, tag="mm")
                    for ri, sb in enumerate(pair):
                        r, i = sb // NK, sb % NK
                        nc.tensor.matmul(
                            ps2[:, ri, :],
                            kT[:, r, P * i:P * (i + 1)],
                            qT[:, TJ * j:TJ * (j + 1)],
                            start=True, stop=True,
                        )
                    e2 = e_pool.tile([P, 2, TJ], bf16, tag="e2")
                    nc.scalar.activation(out=e2, in_=ps2, func=Exp,
                                         scale=INV_SCALE)
                    for ri, sb in enumerate(pair):
                        nc.vector.tensor_mul(
                            out=e2[:, ri, :], in0=e2[:, ri, :],
                            in1=maskt[:, NSB * j + sb, :],
                        )
                    return e2

                def emit_av(pair, e2, mm):
                    for ri, sb in enumerate(pair):
                        r, i = sb // NK, sb % NK
                        st, sp = (mm == 0), (mm == nmm - 1)
                        nc.tensor.matmul(ps_od[:, 0, :], vN[:, r, i, :],
                                         e2[:, ri, :], start=st, stop=sp)
                        nc.tensor.matmul(ps_od[0:1, 1, :], ones_bf,
                                         e2[:, ri, :], start=st, stop=sp)
                        mm += 1
                    return mm

                pairs = [(pi, pi + 1) for pi in range(0, NSB, 2)]
                mm = 0
                prev = None
                for pair in pairs:
                    e2 = emit_scores(pair)
                    if prev is not None:
                        mm = emit_av(prev[0], prev[1], mm)
                    prev = (pair, e2)
                mm = emit_av(prev[0], prev[1], mm)
                oT[j] = stage.tile([P, TJ], f32, tag=f"oT{j}", name=f"oT{j}")
                nc.vector.tensor_copy(out=oT[j], in_=ps_od[:, 0, :])
                nc.vector.tensor_copy(out=denom[0:1, TJ * j:TJ * (j + 1)],
                                      in_=ps_od[0:1, 1, :])

            recip = singles.tile([1, TOWN], f32, tag="recip")
            obounce = dram.tile([TOWN + 16, H], i8)
            sout = singles.tile([P, 8], bf16, tag="sout")

            def out_phase(j):
                rj = recip[0:1, TJ * j:TJ * (j + 1)]
                nc.vector.reciprocal(out=rj,
                                     in_=denom[0:1, TJ * j:TJ * (j + 1)])
                ps = pp_mm.tile([P, 2, TJ], f32, tag="mm")
                nc.tensor.matmul(ps[:, 0, :], ones_row, rj,
                                 start=True, stop=True)
                otn = stage.tile([P, TJ], bf16, tag="otn")
                nc.vector.tensor_mul(out=otn, in0=oT[j], in1=ps[:, 0, :])
                ps_t = pp_tr.tile([P, 2, TJ], bf16, tag="tr")
                for di in range(4):
                    nc.tensor.transpose(
                        ps_t[:, 0, P * di:P * (di + 1)],
                        otn[:, P * di:P * (di + 1)],
                        ident,
                    )
                ob = stage.tile([P, 4, H], bf16, tag="ob")
                nc.vector.tensor_copy(
                    out=ob,
                    in_=ps_t[:, 0, :].rearrange("p (d h) -> p d h", d=4))
                # int8-quantize per token (partition = token): scale=absmax/127
                am = stage.tile([P, 4], f32, tag="am")
                for di in range(4):
                    nc.vector.tensor_reduce(
                        out=am[:, di:di + 1], in_=ob[:, di, :],
                        axis=mybir.AxisListType.X, op=mybir.AluOpType.max,
                        apply_absolute_value=True)
                nc.vector.tensor_scalar(
                    out=am, in0=am, scalar1=1.0 / 127.0, scalar2=1e-30,
                    op0=mybir.AluOpType.mult, op1=mybir.AluOpType.max)
                sc_j = sout[:, 4 * j:4 * (j + 1)]
                nc.vector.tensor_copy(out=sc_j, in_=am)
                sc_f = stage.tile([P, 4], f32, tag="sc_f")
                nc.vector.tensor_copy(out=sc_f, in_=sc_j)
                inv = stage.tile([P, 4], f32, tag="inv")
                nc.vector.reciprocal(out=inv, in_=sc_f)
                qo = stage.tile([P, 4, H], i8, tag="qo")
                for di in range(4):
                    nc.vector.tensor_scalar(
                        out=qo[:, di, :], in0=ob[:, di, :],
                        scalar1=inv[:, di:di + 1], scalar2=None,
                        op0=mybir.AluOpType.mult)
                nc.sync.dma_start(
                    out=obounce[TJ * j:TJ * (j + 1), :].rearrange(
                        "(d p) h -> p d h", p=P),
                    in_=qo,
                )

            attention(0)
            out_phase(0)
            attention(1)
            out_phase(1)
            nc.scalar.dma_start(out=obounce[TOWN:TOWN + 16, :],
                                in_=sout.bitcast(i8))

            # ---- replicate outputs: all-8 AllGather -> out ----
            gout = dram.tile([8, TOWN + 16, H], i8)
            nc.gpsimd.collective_compute(
                "AllGather", mybir.AluOpType.bypass,
                replica_groups=ALL8, ins=[obounce.opt()], outs=[gout.opt()],
            )
            nc.sync.dma_start(
                out=out,
                in_=gout.rearrange("c t h -> (c t) h"),
            )

    nc.compile()
    return nc


def _get_nc():
    if "nc" not in _CACHE:
        _CACHE["nc"] = _build_nc()
    return _CACHE["nc"]


def _thresholds():
    """negc[c, m]: mask threshold per core c, combo m = 16*j + sb."""
    negc = np.zeros((8, NJ * NSB), dtype=np.float32)
    for c in range(8):
        g = c % 2
        for j in range(NJ):
            for sb in range(NSB):
                negc[c, NSB * j + sb] = 128 * sb - 1024 * g - 512 * j
    return negc


def _f32_to_bf16_u16(a):
    """Round-half-up fp32 -> bf16, returned as uint16 payload."""
    u = np.ascontiguousarray(a, dtype=np.float32).view(np.uint32)
    return ((u + 0x8000) >> 16).astype(np.uint16)


def _bf16_u16_to_f32(u):
    return (u.astype(np.uint32) << 16).view(np.float32)


def _get_packer():
    """Cached numpy int8 quantizer: per-token bf16 scales, preallocated
    buffers (host has a single CPU, so minimize passes/allocations)."""
    if "packer" in _CACHE:
        return _CACHE["packer"]

    tmp = np.empty((8, TOWN, C), np.float32)
    q = np.empty((8, TOWN, C), np.int8)
    s = np.empty((8, TOWN, 1), np.float32)
    lo = np.empty((8, TOWN, 1), np.float32)
    inv = np.empty((8, TOWN, 1), np.float32)

    def packer(x8):
        # per-row absmax as max(max(x), -min(x)): two reads, no 32MB write
        np.max(x8, axis=-1, keepdims=True, out=s)
        np.min(x8, axis=-1, keepdims=True, out=lo)
        np.negative(lo, out=lo)
        np.maximum(s, lo, out=s)
        np.divide(s, 127.0, out=s)
        np.maximum(s, 1e-30, out=s)
        u = s.view(np.uint32)
        np.bitwise_and(u + 0x8000, 0xFFFF0000, out=u)  # round scale to bf16
        np.divide(1.0, s, out=inv)
        np.multiply(x8, inv, out=tmp)
        np.rint(tmp, out=tmp)
        np.copyto(q, tmp, casting="unsafe")
        # transposed scale layout: scl_t[c, p*8 + i] = scale[c, 128*i + p]
        st = np.ascontiguousarray(
            (u >> 16).astype(np.uint16).reshape(8, 8, P).transpose(0, 2, 1)
        ).reshape(8, TOWN)
        return q, st

    _CACHE["packer"] = packer
    return packer


def _get_runner():
    """Cached jit(shard_map(bass_exec)) mirroring run_bass_kernel_spmd's
    axon redirect, without per-call re-tracing or donated zero outputs."""
    if "runner" in _CACHE:
        return _CACHE["runner"]

    import jax
    import concourse.mybir as mybir
    from concourse.bass2jax import (
        _bass_exec_p, install_neuronx_cc_hook, partition_id_tensor,
    )
    from jax.sharding import Mesh, PartitionSpec
    from jax.experimental.shard_map import shard_map

    nc = _get_nc()
    install_neuronx_cc_hook()

    partition_name = (nc.partition_id_tensor.name
                      if nc.partition_id_tensor else None)
    in_names, out_names, out_avals = [], [], []
    for alloc in nc.m.functions[0].allocations:
        if not isinstance(alloc, mybir.MemoryLocationSet):
            continue
        name = alloc.memorylocations[0].name
        if alloc.kind == "ExternalInput":
            if name != partition_name:
                in_names.append(name)
        elif alloc.kind == "ExternalOutput":
            out_names.append(name)
            out_avals.append(jax.core.ShapedArray(
                tuple(alloc.tensor_shape), mybir.dt.np(alloc.dtype)))
    assert sorted(in_names) == ["meta", "xq"] and out_names == ["out"], (
        in_names, out_names)
    n_params = len(in_names)
    in_names_all = list(in_names)
    if partition_name is not None:
        in_names_all.append(partition_name)

    def _body(*args):
        operands = list(args)
        if partition_name is not None:
            operands.append(partition_id_tensor())
        return tuple(_bass_exec_p.bind(
            *operands,
            out_avals=tuple(out_avals),
            in_names=tuple(in_names_all),
            out_names=tuple(out_names),
            lowering_input_output_aliases=(),
            sim_require_finite=True,
            sim_require_nnan=True,
            nc=nc,
        ))

    devices = jax.devices()[:8]
    assert len(devices) == 8, f"need 8 devices, have {len(jax.devices())}"
    mesh = Mesh(np.asarray(devices), ("core",))
    sharded = jax.jit(shard_map(
        _body, mesh=mesh,
        in_specs=(PartitionSpec("core"),) * n_params,
        out_specs=(PartitionSpec(),) * len(out_names),
        check_rep=False,
    ))
    _CACHE["runner"] = sharded
    _CACHE["runner_in_names"] = in_names
    return sharded


def kernel(x, Wq, Wk, Wv, mask=None):
    runner = _get_runner()
    packer = _get_packer()

    if "meta" not in _CACHE:
        meta = np.zeros((8, METAR, TOWN), dtype=np.uint8)
        thr8 = _f32_to_bf16_u16(_thresholds()).view(np.uint8)  # [8, 64]
        meta[:, WROWS + 3, 0:2 * NJ * NSB] = thr8
        _CACHE["meta"] = meta
        _CACHE["qo_f"] = np.empty((8, TOWN, H), np.float32)
    meta = _CACHE["meta"]

    x = np.ascontiguousarray(np.asarray(x, dtype=np.float32))
    q, st = packer(x.reshape(8, TOWN, TOWN))

    # int8 weights, per-column bf16 scale
    wall = np.concatenate(
        [np.asarray(Wq, np.float32), np.asarray(Wk, np.float32),
         np.asarray(Wv, np.float32)], axis=1)
    ws = np.abs(wall).max(axis=0, keepdims=True) / 127.0
    wu = ws.view(np.uint32)
    np.bitwise_and(wu + 0x8000, 0xFFFF0000, out=wu)  # round scale to bf16
    wq8 = np.rint(wall / ws).astype(np.int8)
    meta[:, 0:WROWS, :] = wq8.reshape(8, WROWS, TOWN).view(np.uint8)
    meta[:, WROWS, 0:6 * H] = (wu >> 16).astype(np.uint16).view(np.uint8)
    meta[:, WROWS + 1:WROWS + 3, :] = st.view(np.uint8).reshape(8, 2, TOWN)

    args = {
        "xq": q.reshape(8 * TOWN, C),
        "meta": meta.reshape(8 * METAR, TOWN).view(np.int8),
    }
    in_names = _CACHE["runner_in_names"]
    (out_arr,) = runner(*[args[n] for n in in_names])
    ob = np.asarray(out_arr).reshape(8, TOWN + 16, H)
    qo_f = _CACHE["qo_f"]
    np.copyto(qo_f, ob[:, 0:TOWN, :], casting="unsafe")
    # scales: [8, 16, 128] int8 -> uint16 payload [8, 128, 8] -> per-token
    sc_u = np.ascontiguousarray(ob[:, TOWN:TOWN + 16, :]).reshape(
        8, 2048).view(np.uint16).reshape(8, P, 8)
    sc = _bf16_u16_to_f32(np.ascontiguousarray(
        sc_u.reshape(8, P, 2, 4).transpose(0, 2, 3, 1)).reshape(8, TOWN))
    # fresh output each call — callers may hold onto previous results
    return (qo_f * sc[:, :, None]).reshape(B, T, H)
